# revision 1
# baseline (speedup 1.0000x reference)
"""Trainium2 Bass kernel for the KGTM-style GRU message-passing GNN.

Reference math (per time step, T=3):
    agg_in  = A_in  @ nodes          (per batch)
    agg_out = A_in.T @ nodes
    zv = sigmoid(agg_in@W3wa.T + agg_out@W3wb.T + fn@W3u.T)
    rv = sigmoid(agg_in@W4wa.T + agg_out@W4wb.T + fn@W4u.T)
    hv = tanh   (agg_in@W5wa.T + agg_out@W5wb.T + (rv*fn)@W5u.T)
    fn' = fn + zv*(hv - fn)
    out_t = fn'@Wouta.T + x@Woutb.T + b_out

Mapping: pure data parallel over batch (8 cores x 256 batches, padded to 258
= 43 tiles of 6).  On-chip layout "L2" puts (batch-local, channel) on the
128-partition axis (6*20 = 120 partitions) and the node index n (512) on the
free axis.  Aggregation consumes nodes in layout "L1" [m, (b,h)] as the
matmul stationary operand so its output lands directly in L2:
    agg_L2[(b,h), n] = sum_m nodes_L1[m, (b,h)] * A~[m, n].
GRU gate matmuls use block-diagonal weights kron(I6, W.T) [120,120].  A PE
transpose converts fn' back to L1 for the next step's aggregation.  All
matmuls run as float32r (1 row/cycle, ~1.5e-4 rel err).
"""

import numpy as np

import concourse.bacc as bacc
import concourse.tile as tile
import concourse.mybir as mybir
from concourse.bass_utils import run_bass_kernel_spmd

F32 = mybir.dt.float32
F32R = mybir.dt.float32r

B, N, H, T = 2048, 512, 20, 3
NCORES = 8
BS = B // NCORES          # 256 batches per core
BPER = 6                  # batches per partition tile
TP = BPER * H             # 120 partitions per tile
NT = 43                   # tiles per core (43*6 = 258, 2 batches of zero pad)
BPAD = NT * BPER          # 258
MK = N // 128             # 4 contraction chunks of 128 along m

LAST_RESULTS = None       # stash of the most recent BassKernelResults


def _r(ap):
    return ap.bitcast(F32R)


def build_nc():
    nc = bacc.Bacc("TRN2", target_bir_lowering=False, debug=False,
                   num_devices=NCORES)

    xl1_d = nc.dram_tensor("xl1", [NT, 128, MK, TP], F32, kind="ExternalInput")
    xl2_d = nc.dram_tensor("xl2", [NT, TP, N], F32, kind="ExternalInput")
    ain_t_d = nc.dram_tensor("ain_t", [N, N], F32, kind="ExternalInput")
    ain_d = nc.dram_tensor("ain", [N, N], F32, kind="ExternalInput")
    wnames = ["wz_in", "wz_out", "wz_fn", "wr_in", "wr_out", "wr_fn",
              "wh_in", "wh_out", "wh_fn", "wo_fn", "wo_x"]
    w_d = {w: nc.dram_tensor(w, [TP, TP], F32, kind="ExternalInput")
           for w in wnames}
    bias_d = nc.dram_tensor("bias", [TP, 1], F32, kind="ExternalInput")
    ident_d = nc.dram_tensor("ident", [128, 128], F32, kind="ExternalInput")
    out_d = nc.dram_tensor("out", [T, NT, TP, N], F32, kind="ExternalOutput")

    AF = mybir.ActivationFunctionType
    with tile.TileContext(nc) as tc:
        with (
            tc.tile_pool(name="const", bufs=1) as cpool,
            tc.tile_pool(name="io", bufs=3) as iopool,
            tc.tile_pool(name="work", bufs=4) as wpool,
            tc.tile_pool(name="state", bufs=3) as spool,
            tc.tile_pool(name="psA", bufs=1, space="PSUM") as psA,
            tc.tile_pool(name="psB", bufs=1, space="PSUM") as psB,
        ):
            # ---- constants ----
            at_sb = cpool.tile([128, MK, N], F32R, name="at_sb")   # A_in.T rows
            a_sb = cpool.tile([128, MK, N], F32R, name="a_sb")     # A_in rows
            for k in range(MK):
                nc.sync.dma_start(at_sb[:, k, :], ain_t_d.ap()[128 * k:128 * (k + 1), :].bitcast(F32R))
                nc.sync.dma_start(a_sb[:, k, :], ain_d.ap()[128 * k:128 * (k + 1), :].bitcast(F32R))
            w_sb = {}
            for w in wnames:
                w_sb[w] = cpool.tile([TP, TP], F32R, name=f"{w}_sb")
                nc.sync.dma_start(w_sb[w][:], w_d[w].ap().bitcast(F32R))
            bias_sb = cpool.tile([TP, 1], F32, name="bias_sb")
            nc.sync.dma_start(bias_sb[:], bias_d.ap())
            ident = cpool.tile([128, 128], F32R, name="ident")
            nc.sync.dma_start(ident[:], ident_d.ap().bitcast(F32R))

            # ---- per-tile pipeline, emitted as a 3-deep wavefront ----
            # Wave w emits (i=w, t=0), (i=w-1, t=1), (i=w-2, t=2) so every
            # engine's FIFO interleaves three independent tile chains.
            st = [dict() for _ in range(NT)]

            def emit_step(i, t):
                if t == 0:
                    xl1_sb = iopool.tile([128, MK, TP], F32R, name="xl1_sb")
                    nc.sync.dma_start(xl1_sb[:], xl1_d.ap()[i].bitcast(F32R))
                    xl2_sb = iopool.tile([TP, N], F32R, name="xl2_sb", bufs=4)
                    nc.sync.dma_start(xl2_sb[:], xl2_d.ap()[i].bitcast(F32R))
                    st[i]["xl1"] = xl1_sb
                    st[i]["xl2"] = xl2_sb
                    st[i]["fn"] = xl2_sb      # step-0 node state is x itself
                    # skip-connection projection of x is step-invariant
                    ox_ps = psB.tile([TP, N], F32, name="ox_ps")
                    nc.tensor.matmul(ox_ps[:], w_sb["wo_x"][:], xl2_sb[:],
                                     start=True, stop=True)
                    ox_sb = wpool.tile([TP, N], F32, name="ox_sb", bufs=4)
                    nc.vector.tensor_copy(ox_sb[:], ox_ps[:])
                    st[i]["ox"] = ox_sb
                xl1_sb = st[i]["xl1"]
                xl2_sb = st[i]["xl2"]
                fn_sb = st[i]["fn"]
                fnl1_sb = st[i].get("fnl1")
                ox_sb = st[i]["ox"]
                if True:
                    # aggregation: agg = nodes_L1.T @ A~  -> L2 layout
                    agg_in_ps = psA.tile([TP, N], F32, name="agg_in_ps")
                    agg_out_ps = psA.tile([TP, N], F32, name="agg_out_ps")
                    lhs = xl1_sb if t == 0 else fnl1_sb
                    for k in range(MK):
                        nc.tensor.matmul(agg_in_ps[:], lhs[:, k, :],
                                         at_sb[:, k, :],
                                         start=(k == 0), stop=(k == MK - 1))
                    for k in range(MK):
                        nc.tensor.matmul(agg_out_ps[:], lhs[:, k, :],
                                         a_sb[:, k, :],
                                         start=(k == 0), stop=(k == MK - 1))
                    agg_in_sb = wpool.tile([TP, N], F32R, name="agg_in_sb")
                    agg_out_sb = wpool.tile([TP, N], F32R, name="agg_out_sb")
                    nc.scalar.copy(agg_in_sb[:], agg_in_ps[:])
                    nc.scalar.copy(agg_out_sb[:], agg_out_ps[:])

                    # gates: z and r share one 2-bank psum tile -> one sigmoid
                    zr_ps = psB.tile([TP, 2, N], F32, name="zr_ps")
                    nc.tensor.matmul(zr_ps[:, 0, :], w_sb["wz_in"][:], agg_in_sb[:], start=True, stop=False)
                    nc.tensor.matmul(zr_ps[:, 0, :], w_sb["wz_out"][:], agg_out_sb[:], start=False, stop=False)
                    nc.tensor.matmul(zr_ps[:, 0, :], w_sb["wz_fn"][:], fn_sb[:], start=False, stop=True)
                    nc.tensor.matmul(zr_ps[:, 1, :], w_sb["wr_in"][:], agg_in_sb[:], start=True, stop=False)
                    nc.tensor.matmul(zr_ps[:, 1, :], w_sb["wr_out"][:], agg_out_sb[:], start=False, stop=False)
                    nc.tensor.matmul(zr_ps[:, 1, :], w_sb["wr_fn"][:], fn_sb[:], start=False, stop=True)
                    zr_sb = wpool.tile([TP, 2, N], F32, name="zr_sb")
                    nc.scalar.activation(zr_sb[:], zr_ps[:], AF.Sigmoid)
                    z_sb = zr_sb[:, 0, :]
                    r_sb = zr_sb[:, 1, :]
                    rf_sb = wpool.tile([TP, N], F32R, name="rf_sb")
                    nc.vector.tensor_mul(rf_sb[:], r_sb, fn_sb[:].bitcast(F32))
                    # zf1 = (z-1)*fn, off the tanh critical path (GpSimd)
                    zf1_sb = wpool.tile([TP, N], F32, name="zf1_sb")
                    nc.vector.scalar_tensor_tensor(
                        zf1_sb[:], z_sb, 1.0, fn_sb[:].bitcast(F32),
                        op0=mybir.AluOpType.subtract, op1=mybir.AluOpType.mult)

                    h_ps = psB.tile([TP, N], F32, name="h_ps")
                    nc.tensor.matmul(h_ps[:], w_sb["wh_in"][:], agg_in_sb[:], start=True, stop=False)
                    nc.tensor.matmul(h_ps[:], w_sb["wh_out"][:], agg_out_sb[:], start=False, stop=False)
                    nc.tensor.matmul(h_ps[:], w_sb["wh_fn"][:], rf_sb[:], start=False, stop=True)
                    h_sb = wpool.tile([TP, N], F32, name="h_sb")
                    nc.scalar.activation(h_sb[:], h_ps[:], AF.Tanh)

                    # fn' = fn + z*(h - fn) = z*h - (z-1)*fn
                    zh_sb = wpool.tile([TP, N], F32, name="zh_sb")
                    nc.vector.tensor_mul(zh_sb[:], z_sb, h_sb[:])
                    fnn_sb = spool.tile([TP, N], F32R, name="fnn_sb", bufs=4)
                    nc.vector.tensor_sub(fnn_sb[:], zh_sb[:], zf1_sb[:])

                    # output projection: o = wo_fn@fn' + (hoisted x part) + bias
                    o_ps = psB.tile([TP, N], F32, name="o_ps")
                    nc.tensor.matmul(o_ps[:], w_sb["wo_fn"][:], fnn_sb[:], start=True, stop=True)
                    o_sb = iopool.tile([TP, N], F32, name="o_sb")
                    nc.vector.scalar_tensor_tensor(
                        o_sb[:], o_ps[:], bias_sb[:], ox_sb[:],
                        op0=mybir.AluOpType.add, op1=mybir.AluOpType.add)
                    nc.sync.dma_start(out_d.ap()[t, i], o_sb[:])

                    # transpose fn' into L1 for the next step's aggregation
                    if t < T - 1:
                        tp_ps = psA.tile([128, MK, TP], F32R, name="tp_ps")
                        fnl1_sb = spool.tile([128, MK, TP], F32R, name="fnl1_sb", bufs=4)
                        for k in range(MK):
                            nc.tensor.transpose(
                                tp_ps[:, k, :],
                                fnn_sb[:, 128 * k:128 * (k + 1)],
                                ident[0:TP, 0:TP])
                        nc.scalar.copy(fnl1_sb[:], tp_ps[:])
                        st[i]["fnl1"] = fnl1_sb
                    st[i]["fn"] = fnn_sb

            for w in range(NT + T - 1):
                for t in range(T):
                    i = w - t
                    if 0 <= i < NT:
                        emit_step(i, t)

    nc.compile()
    return nc


_NC_CACHE = None


def _get_nc():
    global _NC_CACHE
    if _NC_CACHE is None:
        _NC_CACHE = build_nc()
    return _NC_CACHE


def _host_prep(x, A_in, W3w, W3u, W4w, W4u, W5w, W5u, W_out, b_out):
    f32 = np.float32
    eye = np.eye(BPER, dtype=f32)

    def blk(w):
        return np.ascontiguousarray(np.kron(eye, np.asarray(w, f32).T))

    shared = {
        "ain_t": np.ascontiguousarray(np.asarray(A_in, f32).T),
        "ain": np.ascontiguousarray(np.asarray(A_in, f32)),
        "wz_in": blk(W3w[:, :H]), "wz_out": blk(W3w[:, H:]), "wz_fn": blk(W3u),
        "wr_in": blk(W4w[:, :H]), "wr_out": blk(W4w[:, H:]), "wr_fn": blk(W4u),
        "wh_in": blk(W5w[:, :H]), "wh_out": blk(W5w[:, H:]), "wh_fn": blk(W5u),
        "wo_fn": blk(W_out[:, :H]), "wo_x": blk(W_out[:, H:]),
        "bias": np.ascontiguousarray(
            np.tile(np.asarray(b_out, f32), BPER)[:, None]),
        "ident": np.eye(128, dtype=f32),
    }

    in_maps = []
    x = np.asarray(x, f32)
    for c in range(NCORES):
        xp = np.zeros((BPAD, N, H), f32)
        xp[:BS] = x[BS * c:BS * (c + 1)]
        # L1: [m, (b,h)] -> dram [NT, 128(p), MK(k), TP(j)], m = 128k+p
        l1 = xp.transpose(1, 0, 2).reshape(N, NT, TP).transpose(1, 0, 2)
        l1 = l1.reshape(NT, MK, 128, TP).transpose(0, 2, 1, 3)
        # L2: [(b,h), n] -> dram [NT, TP, N]
        l2 = xp.transpose(0, 2, 1).reshape(NT, TP, N)
        in_maps.append({"xl1": np.ascontiguousarray(l1),
                        "xl2": np.ascontiguousarray(l2), **shared})
    return in_maps


def kernel(x, A_in, W3w, W3u, W4w, W4u, W5w, W5u, W_out, b_out):
    global LAST_RESULTS
    nc = _get_nc()
    in_maps = _host_prep(x, A_in, W3w, W3u, W4w, W4u, W5w, W5u, W_out, b_out)
    res = run_bass_kernel_spmd(nc, in_maps, core_ids=list(range(NCORES)))
    LAST_RESULTS = res
    outs = []
    for c in range(NCORES):
        o = res.results[c]["out"]                      # [T, NT, TP, N]
        o = o.reshape(T, NT, BPER, H, N).transpose(0, 1, 2, 4, 3)
        outs.append(o.reshape(T, BPAD, N, H)[:, :BS])  # drop pad batches
    return np.ascontiguousarray(np.concatenate(outs, axis=1))



# revision 5
# speedup vs baseline: 1.2123x; 1.2123x over previous
"""Trainium2 Bass kernel for the KGTM-style GRU message-passing GNN.

Reference math (per time step, T=3):
    agg_in  = A_in  @ nodes          (per batch)
    agg_out = A_in.T @ nodes
    zv = sigmoid(agg_in@W3wa.T + agg_out@W3wb.T + fn@W3u.T)
    rv = sigmoid(agg_in@W4wa.T + agg_out@W4wb.T + fn@W4u.T)
    hv = tanh   (agg_in@W5wa.T + agg_out@W5wb.T + (rv*fn)@W5u.T)
    fn' = (1-zv)*fn + zv*hv = hv - q*(hv - fn)   with q = 1-zv
    out_t = fn'@Wouta.T + x@Woutb.T + b_out

Mapping: pure data parallel over batch (8 cores x 256 batches, padded to 258
= 43 tiles of 6).  On-chip layout "L2" puts (batch-local, channel) on the
128-partition axis (6*20 = 120 partitions) and the node index n (512) on the
free axis; layout "L1" is the transpose ([node m, (b,h)]), used as the
stationary operand of the aggregation so agg lands directly in L2.

Precision/engine scheme (cost-model driven):
  - Aggregation + z/r/h matmuls run as fp8e4 DoubleRow (2 k-tiles per mm,
    0.5 cycles/row).  Scales: A*256, agg-cast*(16/256), gate weights *32,
    fn-side weights *512; the sigmoid/tanh activation descales by 1/512.
    r's stationaries are negated so one fused sigmoid over (q|r) with
    scale=-1/512 yields q=1-z and r.
  - The output projection runs in bf16 (direct output path needs accuracy).
  - DVE does the bf16 state chain (hmf=h-fn, m=q*hmf, fn'=h-m) at 2x rate
    plus the fp8 recast of fn'; Pool (gpsimd) does the agg fp8 cast; Act
    does sigmoid/tanh/transpose-evac/ox-evac.
  - fn' returns to L1 for the next aggregation via 4 fp8 PE transposes.
"""

import numpy as np
import ml_dtypes

import concourse.bacc as bacc
import concourse.tile as tile
import concourse.mybir as mybir
from concourse.bass_utils import run_bass_kernel_spmd

F32 = mybir.dt.float32
BF16 = mybir.dt.bfloat16
F16 = mybir.dt.float16
FP8 = mybir.dt.float8e4
AF = mybir.ActivationFunctionType
ALU = mybir.AluOpType
DR = mybir.MatmulPerfMode.DoubleRow

E4NP = ml_dtypes.float8_e4m3
BFNP = ml_dtypes.bfloat16

B, N, H, T = 2048, 512, 20, 3
NCORES = 8
BS = B // NCORES          # 256 batches per core
BPER = 6                  # batches per partition tile
TP = BPER * H             # 120 partitions per tile
NT = 43                   # tiles per core (43*6 = 258, 2 batches of zero pad)
BPAD = NT * BPER          # 258
MK = N // 128             # 4 m-chunks of 128

SA = 256.0                # A scale
SAGG = 16.0               # agg fp8 scale
SW = 32.0                 # gate agg-side weight scale
SG = SAGG * SW            # gate psum scale (fn-side weights use this)

LAST_RESULTS = None


def build_nc():
    nc = bacc.Bacc("TRN2", target_bir_lowering=False, debug=False,
                   num_devices=NCORES)

    xl1_d = nc.dram_tensor("xl1", [NT, 128, MK, TP], FP8, kind="ExternalInput")
    xbf_d = nc.dram_tensor("xbf", [NT, TP + 1, N], BF16, kind="ExternalInput")
    xfp_d = nc.dram_tensor("xfp", [NT, TP, N], FP8, kind="ExternalInput")
    atk_d = nc.dram_tensor("atk", [128, MK, N], FP8, kind="ExternalInput")
    ak_d = nc.dram_tensor("ak", [128, MK, N], FP8, kind="ExternalInput")
    # fp8 DoubleRow gate stationaries [K=120, 2, M=120]
    w8names = ["wz_ag", "wr_ag", "wh_ag", "wz_fn", "wr_fn"]
    w8_d = {w: nc.dram_tensor(w, [TP, 2, TP], FP8, kind="ExternalInput")
            for w in w8names}
    woa_d = nc.dram_tensor("woa", [TP, TP], BF16, kind="ExternalInput")
    wob_d = nc.dram_tensor("wob", [TP + 1, TP], BF16, kind="ExternalInput")
    whu_d = nc.dram_tensor("whu", [TP, TP], BF16, kind="ExternalInput")
    ident_d = nc.dram_tensor("ident", [128, 128], FP8, kind="ExternalInput")
    out_d = nc.dram_tensor("out", [NT, TP, T, N], F16, kind="ExternalOutput")

    with tile.TileContext(nc) as tc:
        with (
            tc.tile_pool(name="const", bufs=1) as cpool,
            tc.tile_pool(name="state", bufs=4) as spool,
            tc.tile_pool(name="work", bufs=3) as wpool,
            tc.tile_pool(name="psA", bufs=1, space="PSUM") as psA,
            tc.tile_pool(name="psB", bufs=1, space="PSUM") as psB,
            tc.tile_pool(name="psO", bufs=2, space="PSUM") as psO,
        ):
            # ---- constants ----
            atk = cpool.tile([128, MK, N], FP8, name="atk")
            ak = cpool.tile([128, MK, N], FP8, name="ak")
            nc.sync.dma_start(atk[:], atk_d.ap())
            nc.sync.dma_start(ak[:], ak_d.ap())
            w8 = {}
            for w in w8names:
                w8[w] = cpool.tile([TP, 2, TP], FP8, name=f"{w}_sb")
                nc.sync.dma_start(w8[w][:], w8_d[w].ap())
            woa = cpool.tile([TP, TP], BF16, name="woa")
            wob = cpool.tile([TP + 1, TP], BF16, name="wob")
            whu = cpool.tile([TP, TP], BF16, name="whu")
            nc.sync.dma_start(woa[:], woa_d.ap())
            nc.sync.dma_start(wob[:], wob_d.ap())
            nc.sync.dma_start(whu[:], whu_d.ap())
            ident = cpool.tile([128, 128], FP8, name="ident")
            nc.sync.dma_start(ident[:], ident_d.ap())

            st = [dict() for _ in range(NT)]

            def emit_step(i, t):
                if t == 0:
                    xl1 = spool.tile([128, MK, TP], FP8, name="xl1_sb")
                    nc.sync.dma_start(xl1[:], xl1_d.ap()[i])
                    fnb = spool.tile([TP + 1, N], BF16, name="fnb_sb")
                    nc.sync.dma_start(fnb[:], xbf_d.ap()[i])
                    fr = spool.tile([TP, 2, N], FP8, name="fr_sb")
                    nc.sync.dma_start(fr[:, 0, :], xfp_d.ap()[i])
                    nc.sync.dma_start(fr[:, 1, :], xfp_d.ap()[i])
                    osb = spool.tile([TP, T, N], F16, name="osb_sb")
                    st[i].update(xl1=xl1, fnb=fnb, fr=fr, osb=osb)
                    # hoisted skip projection: ox = Wout_b@x + b_out (bf16)
                    ox_ps = psO.tile([TP, N], F32, name="ox_ps", tag="o")
                    nc.tensor.matmul(ox_ps[:], wob[:], fnb[:],
                                     start=True, stop=True)
                    ox = spool.tile([TP, N], F32, name="ox_sb")
                    nc.scalar.copy(ox[:], ox_ps[:])
                    st[i]["ox"] = ox

                xl1 = st[i]["xl1"]
                fnb = st[i]["fnb"]
                fr = st[i]["fr"]
                osb = st[i]["osb"]
                ox = st[i]["ox"]
                fnl1 = st[i].get("fnl1")
                lhs = xl1 if t == 0 else fnl1

                # aggregation (DoubleRow over m): agg_ps[:,0,:] = in-dir,
                # [:,1,:] = out-dir; psum scale SA
                agg_ps = psA.tile([TP, 2, N], F32, name="agg_ps", tag="agg")
                for k2 in range(2):
                    nc.tensor.matmul(agg_ps[:, 0, :], lhs[:, 2*k2:2*k2+2, :],
                                     atk[:, 2*k2:2*k2+2, :], perf_mode=DR,
                                     start=(k2 == 0), stop=(k2 == 1))
                for k2 in range(2):
                    nc.tensor.matmul(agg_ps[:, 1, :], lhs[:, 2*k2:2*k2+2, :],
                                     ak[:, 2*k2:2*k2+2, :], perf_mode=DR,
                                     start=(k2 == 0), stop=(k2 == 1))
                # agg cast: fp8 = (SAGG/SA) * psum   (Pool)
                agg = wpool.tile([TP, 2, N], FP8, name="agg_sb")
                nc.gpsimd.tensor_scalar_mul(agg[:], agg_ps[:], SAGG / SA)

                # gates: zr psum (scale SG); r stationaries are negated
                zr_ps = psB.tile([TP, 2, N], F32, name="zr_ps", tag="zr")
                nc.tensor.matmul(zr_ps[:, 0, :], w8["wz_ag"][:], agg[:],
                                 perf_mode=DR, start=True, stop=False)
                nc.tensor.matmul(zr_ps[:, 0, :], w8["wz_fn"][:], fr[:],
                                 perf_mode=DR, start=False, stop=True)
                nc.tensor.matmul(zr_ps[:, 1, :], w8["wr_ag"][:], agg[:],
                                 perf_mode=DR, start=True, stop=False)
                nc.tensor.matmul(zr_ps[:, 1, :], w8["wr_fn"][:], fr[:],
                                 perf_mode=DR, start=False, stop=True)
                # one sigmoid with scale=-1/SG gives (q, r)
                qr = wpool.tile([TP, 2, N], BF16, name="qr_sb")
                nc.scalar.activation(qr[:], zr_ps[:], AF.Sigmoid, scale=-1.0 / SG)

                # h: fp8 agg part + bf16 (r*fn) part, tanh scale 1/SG
                rf = wpool.tile([TP, N], BF16, name="rf_sb")
                nc.vector.tensor_mul(rf[:], qr[:, 1, :], fnb[0:TP, :])
                h_ps = psB.tile([TP, N], F32, name="h_ps", tag="h")
                nc.tensor.matmul(h_ps[:], w8["wh_ag"][:], agg[:],
                                 perf_mode=DR, start=True, stop=False)
                nc.tensor.matmul(h_ps[:], whu[:], rf[:],
                                 start=False, stop=True)
                h = wpool.tile([TP, N], BF16, name="h_sb")
                nc.scalar.activation(h[:], h_ps[:], AF.Tanh, scale=1.0 / SG)

                # state update (bf16, DVE 2x): fn' = h - q*(h - fn)
                hmf = wpool.tile([TP, N], BF16, name="hmf_sb")
                nc.vector.tensor_sub(hmf[:], h[:], fnb[0:TP, :])
                mq = wpool.tile([TP, N], BF16, name="mq_sb")
                nc.vector.tensor_mul(mq[:], qr[:, 0, :], hmf[:])
                nc.vector.tensor_sub(fnb[0:TP, :], h[:], mq[:])
                # fp8 recast of fn' for next matmuls / transposes
                nc.vector.tensor_copy(fr[:, 0, :], fnb[0:TP, :])

                # output: o = Woa@fn' (+ ox) -> f16
                o_ps = psO.tile([TP, N], F32, name="o_ps", tag="o")
                nc.tensor.matmul(o_ps[:], woa[:], fnb[0:TP, :],
                                 start=True, stop=True)
                nc.vector.tensor_add(osb[:, t, :], o_ps[:], ox[:])
                if t == T - 1:
                    nc.sync.dma_start(out_d.ap()[i], osb[:])

                # fn' -> L1 for next aggregation (fp8 PE transposes)
                if t < T - 1:
                    tp_ps = psA.tile([128, MK, TP], FP8, name="tp_ps", tag="tp")
                    fnl1 = spool.tile([128, MK, TP], FP8, name="fnl1_sb")
                    for k in range(MK):
                        nc.tensor.transpose(
                            tp_ps[:, k, :],
                            fr[:, 0, 128*k:128*(k+1)],
                            ident[0:TP, 0:TP])
                    nc.scalar.copy(fnl1[:], tp_ps[:])
                    st[i]["fnl1"] = fnl1

            for w in range(NT + T - 1):
                for t in range(T):
                    i = w - t
                    if 0 <= i < NT:
                        emit_step(i, t)

    nc.compile()
    return nc


_NC_CACHE = None


def _get_nc():
    global _NC_CACHE
    if _NC_CACHE is None:
        _NC_CACHE = build_nc()
    return _NC_CACHE


def _q8(x, scale=1.0):
    return np.clip(np.asarray(x, np.float32) * scale, -240, 240).astype(E4NP)


def _kron6(w):
    return np.kron(np.eye(BPER, dtype=np.float32), np.asarray(w, np.float32).T)


def _host_prep(x, A_in, W3w, W3u, W4w, W4u, W5w, W5u, W_out, b_out):
    f32 = np.float32
    A_in = np.asarray(A_in, f32)

    def achunks(a):  # [N,N] -> [128, MK, N], m = 128*k + p, fp8 scaled
        return np.ascontiguousarray(
            _q8(a, SA).reshape(MK, 128, N).transpose(1, 0, 2))

    def pair8(wa, wb, s):  # [K,2,M] fp8 DoubleRow stationary
        p = np.stack([_kron6(wa), _kron6(wb)], axis=1)   # [120, 2, 120]
        return np.ascontiguousarray(_q8(p, s))

    zeros = np.zeros((H, H), f32)
    shared = {
        "atk": achunks(A_in.T),
        "ak": achunks(A_in),
        "wz_ag": pair8(W3w[:, :H], W3w[:, H:], SW),
        "wr_ag": pair8(-W4w[:, :H], -W4w[:, H:], SW),
        "wh_ag": pair8(W5w[:, :H], W5w[:, H:], SW),
        "wz_fn": pair8(W3u, zeros, SG),
        "wr_fn": pair8(-W4u, zeros, SG),
        "woa": np.ascontiguousarray(_kron6(W_out[:, :H]).astype(BFNP)),
        "wob": np.ascontiguousarray(np.concatenate(
            [_kron6(W_out[:, H:]),
             np.tile(np.asarray(b_out, f32), BPER)[None, :]], axis=0
        ).astype(BFNP)),
        "whu": np.ascontiguousarray((_kron6(W5u) * SG).astype(BFNP)),
        "ident": np.eye(128, dtype=f32).astype(E4NP),
    }

    in_maps = []
    x = np.asarray(x, f32)
    for c in range(NCORES):
        xp = np.zeros((BPAD, N, H), f32)
        xp[:BS] = x[BS * c:BS * (c + 1)]
        # L1: [m, (b,h)] -> [NT, 128(p), MK(k), TP], m = 128k+p
        l1 = xp.transpose(1, 0, 2).reshape(N, NT, TP).transpose(1, 0, 2)
        l1 = l1.reshape(NT, MK, 128, TP).transpose(0, 2, 1, 3)
        # L2: [(b,h), n] -> [NT, TP, N]
        l2 = xp.transpose(0, 2, 1).reshape(NT, TP, N)
        l2e = np.concatenate(
            [l2, np.ones((NT, 1, N), f32)], axis=1)     # ones row for bias
        in_maps.append({
            "xl1": np.ascontiguousarray(_q8(l1)),
            "xbf": np.ascontiguousarray(l2e.astype(BFNP)),
            "xfp": np.ascontiguousarray(_q8(l2)),
            **shared})
    return in_maps


def kernel(x, A_in, W3w, W3u, W4w, W4u, W5w, W5u, W_out, b_out):
    global LAST_RESULTS
    nc = _get_nc()
    in_maps = _host_prep(x, A_in, W3w, W3u, W4w, W4u, W5w, W5u, W_out, b_out)
    res = run_bass_kernel_spmd(nc, in_maps, core_ids=list(range(NCORES)))
    LAST_RESULTS = res
    outs = []
    for c in range(NCORES):
        o = np.asarray(res.results[c]["out"], np.float32)  # [NT, TP, T, N]
        o = o.reshape(NT, BPER, H, T, N).transpose(3, 0, 1, 4, 2)
        outs.append(o.reshape(T, BPAD, N, H)[:, :BS])
    return np.ascontiguousarray(np.concatenate(outs, axis=1))


# revision 6
# speedup vs baseline: 1.4317x; 1.1810x over previous
"""Trainium2 Bass kernel for the KGTM-style GRU message-passing GNN.

Reference math (per time step, T=3):
    agg_in  = A_in  @ nodes          (per batch)
    agg_out = A_in.T @ nodes
    zv = sigmoid(agg_in@W3wa.T + agg_out@W3wb.T + fn@W3u.T)
    rv = sigmoid(agg_in@W4wa.T + agg_out@W4wb.T + fn@W4u.T)
    hv = tanh   (agg_in@W5wa.T + agg_out@W5wb.T + (rv*fn)@W5u.T)
    fn' = (1-zv)*fn + zv*hv = hv - q*(hv - fn)   with q = 1-zv
    out_t = fn'@Wouta.T + x@Woutb.T + b_out

Mapping: pure data parallel over batch (8 cores x 256 batches, padded to 258
= 43 tiles of 6).  On-chip layout "L2" puts (batch-local, channel) on the
128-partition axis (6*20 = 120 partitions) and the node index n (512) on the
free axis; layout "L1" is the transpose ([node m, (b,h)]), used as the
stationary operand of the aggregation so agg lands directly in L2.

Precision/engine scheme (cost-model driven):
  - Aggregation + z/r/h matmuls run as fp8e4 DoubleRow (2 k-tiles per mm,
    0.5 cycles/row).  Scales: A*256, agg-cast*(16/256), gate weights *32,
    fn-side weights *512; the sigmoid/tanh activation descales by 1/512.
    r's stationaries are negated so one fused sigmoid over (q|r) with
    scale=-1/512 yields q=1-z and r.
  - The output projection runs in bf16 (direct output path needs accuracy).
  - DVE does the bf16 state chain (hmf=h-fn, m=q*hmf, fn'=h-m) at 2x rate
    plus the fp8 recast of fn'; Pool (gpsimd) does the agg fp8 cast; Act
    does sigmoid/tanh/transpose-evac/ox-evac.
  - fn' returns to L1 for the next aggregation via 4 fp8 PE transposes.
"""

import numpy as np
import ml_dtypes

import concourse.bacc as bacc
import concourse.tile as tile
import concourse.mybir as mybir
from concourse.bass_utils import run_bass_kernel_spmd

F32 = mybir.dt.float32
BF16 = mybir.dt.bfloat16
F16 = mybir.dt.float16
FP8 = mybir.dt.float8e4
AF = mybir.ActivationFunctionType
ALU = mybir.AluOpType
DR = mybir.MatmulPerfMode.DoubleRow

E4NP = ml_dtypes.float8_e4m3
BFNP = ml_dtypes.bfloat16

B, N, H, T = 2048, 512, 20, 3
NCORES = 8
BS = B // NCORES          # 256 batches per core
BPER = 6                  # batches per partition tile
TP = BPER * H             # 120 partitions per tile
NT = 43                   # tiles per core (43*6 = 258, 2 batches of zero pad)
BPAD = NT * BPER          # 258
MK = N // 128             # 4 m-chunks of 128
WSTRIDE = 4               # waves between consecutive steps of one tile

SA = 256.0                # A scale
SAGG = 16.0               # agg fp8 scale
SW = 32.0                 # gate agg-side weight scale
SG = SAGG * SW            # gate psum scale (fn-side weights use this)

LAST_RESULTS = None


def build_nc():
    nc = bacc.Bacc("TRN2", target_bir_lowering=False, debug=False,
                   num_devices=NCORES)

    xl1_d = nc.dram_tensor("xl1", [NT, 128, MK, TP], FP8, kind="ExternalInput")
    xbf_d = nc.dram_tensor("xbf", [NT, TP + 1, N], BF16, kind="ExternalInput")
    xfp_d = nc.dram_tensor("xfp", [NT, TP, N], FP8, kind="ExternalInput")
    atk_d = nc.dram_tensor("atk", [128, MK, N], FP8, kind="ExternalInput")
    ak_d = nc.dram_tensor("ak", [128, MK, N], FP8, kind="ExternalInput")
    # fp8 DoubleRow gate stationaries [K=120, 2, M=120]
    w8names = ["wz_ag", "wr_ag", "wh_ag", "wz_fn", "wr_fn"]
    w8_d = {w: nc.dram_tensor(w, [TP, 2, TP], FP8, kind="ExternalInput")
            for w in w8names}
    woa_d = nc.dram_tensor("woa", [TP, TP], BF16, kind="ExternalInput")
    wob_d = nc.dram_tensor("wob", [TP + 1, TP], BF16, kind="ExternalInput")
    whu_d = nc.dram_tensor("whu", [TP, TP], BF16, kind="ExternalInput")
    ident_d = nc.dram_tensor("ident", [128, 128], FP8, kind="ExternalInput")
    out_d = nc.dram_tensor("out", [NT, TP, T, N], F16, kind="ExternalOutput")

    with tile.TileContext(nc) as tc:
        with (
            tc.tile_pool(name="const", bufs=1) as cpool,
            tc.tile_pool(name="state", bufs=2 * WSTRIDE + 4) as spool,
            tc.tile_pool(name="work", bufs=4) as wpool,
            tc.tile_pool(name="psA", bufs=1, space="PSUM") as psA,
            tc.tile_pool(name="psB", bufs=1, space="PSUM") as psB,
            tc.tile_pool(name="psO", bufs=2, space="PSUM") as psO,
        ):
            # ---- constants ----
            atk = cpool.tile([128, MK, N], FP8, name="atk")
            ak = cpool.tile([128, MK, N], FP8, name="ak")
            nc.sync.dma_start(atk[:], atk_d.ap())
            nc.sync.dma_start(ak[:], ak_d.ap())
            w8 = {}
            for w in w8names:
                w8[w] = cpool.tile([TP, 2, TP], FP8, name=f"{w}_sb")
                nc.sync.dma_start(w8[w][:], w8_d[w].ap())
            woa = cpool.tile([TP, TP], BF16, name="woa")
            wob = cpool.tile([TP + 1, TP], BF16, name="wob")
            whu = cpool.tile([TP, TP], BF16, name="whu")
            nc.sync.dma_start(woa[:], woa_d.ap())
            nc.sync.dma_start(wob[:], wob_d.ap())
            nc.sync.dma_start(whu[:], whu_d.ap())
            ident = cpool.tile([128, 128], FP8, name="ident")
            nc.sync.dma_start(ident[:], ident_d.ap())

            st = [dict() for _ in range(NT)]

            def emit_step(i, t):
                if t == 0:
                    xl1 = spool.tile([128, MK, TP], FP8, name="xl1_sb")
                    nc.sync.dma_start(xl1[:], xl1_d.ap()[i])
                    fnb = spool.tile([TP + 1, N], BF16, name="fnb_sb")
                    nc.sync.dma_start(fnb[:], xbf_d.ap()[i])
                    fr = spool.tile([TP, 2, N], FP8, name="fr_sb")
                    nc.sync.dma_start(fr[:, 0, :], xfp_d.ap()[i])
                    nc.sync.dma_start(fr[:, 1, :], xfp_d.ap()[i])
                    osb = spool.tile([TP, T, N], F16, name="osb_sb")
                    st[i].update(xl1=xl1, fnb=fnb, fr=fr, osb=osb)
                    # hoisted skip projection: ox = Wout_b@x + b_out (bf16)
                    ox_ps = psO.tile([TP, N], F32, name="ox_ps", tag="o")
                    nc.tensor.matmul(ox_ps[:], wob[:], fnb[:],
                                     start=True, stop=True)
                    ox = spool.tile([TP, N], F32, name="ox_sb")
                    nc.gpsimd.tensor_copy(ox[:], ox_ps[:])
                    st[i]["ox"] = ox

                xl1 = st[i]["xl1"]
                fnb = st[i]["fnb"]
                fr = st[i]["fr"]
                osb = st[i]["osb"]
                ox = st[i]["ox"]
                fnl1 = st[i].get("fnl1")
                lhs = xl1 if t == 0 else fnl1

                # aggregation (DoubleRow over m); separate psum tags so the
                # in/out chains pipeline independently; casts split DVE/Pool
                agi_ps = psA.tile([TP, N], F32, name="agi_ps", tag="agi")
                ago_ps = psA.tile([TP, N], F32, name="ago_ps", tag="ago")
                for k2 in range(2):
                    nc.tensor.matmul(agi_ps[:], lhs[:, 2*k2:2*k2+2, :],
                                     atk[:, 2*k2:2*k2+2, :], perf_mode=DR,
                                     start=(k2 == 0), stop=(k2 == 1))
                for k2 in range(2):
                    nc.tensor.matmul(ago_ps[:], lhs[:, 2*k2:2*k2+2, :],
                                     ak[:, 2*k2:2*k2+2, :], perf_mode=DR,
                                     start=(k2 == 0), stop=(k2 == 1))
                agg = wpool.tile([TP, 2, N], FP8, name="agg_sb")
                nc.vector.tensor_scalar_mul(agg[:, 0, :], agi_ps[:], SAGG / SA)
                nc.gpsimd.tensor_scalar_mul(agg[:, 1, :], ago_ps[:], SAGG / SA)

                # gates: zr psum (scale SG); r stationaries are negated
                zr_ps = psB.tile([TP, 2, N], F32, name="zr_ps", tag="zr")
                nc.tensor.matmul(zr_ps[:, 0, :], w8["wz_ag"][:], agg[:],
                                 perf_mode=DR, start=True, stop=False)
                nc.tensor.matmul(zr_ps[:, 0, :], w8["wz_fn"][:], fr[:],
                                 perf_mode=DR, start=False, stop=True)
                nc.tensor.matmul(zr_ps[:, 1, :], w8["wr_ag"][:], agg[:],
                                 perf_mode=DR, start=True, stop=False)
                nc.tensor.matmul(zr_ps[:, 1, :], w8["wr_fn"][:], fr[:],
                                 perf_mode=DR, start=False, stop=True)
                # one sigmoid with scale=-1/SG gives (q, r)
                qr = wpool.tile([TP, 2, N], BF16, name="qr_sb")
                nc.scalar.activation(qr[:], zr_ps[:], AF.Sigmoid, scale=-1.0 / SG)

                # h: fp8 agg part + bf16 (r*fn) part, tanh scale 1/SG
                rf = wpool.tile([TP, N], BF16, name="rf_sb")
                nc.vector.tensor_mul(rf[:], qr[:, 1, :], fnb[0:TP, :])
                h_ps = psB.tile([TP, N], F32, name="h_ps", tag="h")
                nc.tensor.matmul(h_ps[:], w8["wh_ag"][:], agg[:],
                                 perf_mode=DR, start=True, stop=False)
                nc.tensor.matmul(h_ps[:], whu[:], rf[:],
                                 start=False, stop=True)
                h = wpool.tile([TP, N], BF16, name="h_sb")
                nc.scalar.activation(h[:], h_ps[:], AF.Tanh, scale=1.0 / SG)

                # state update (bf16, DVE 2x): fn' = h - q*(h - fn)
                hmf = wpool.tile([TP, N], BF16, name="hmf_sb")
                nc.vector.tensor_sub(hmf[:], h[:], fnb[0:TP, :])
                mq = wpool.tile([TP, N], BF16, name="mq_sb")
                nc.vector.tensor_mul(mq[:], qr[:, 0, :], hmf[:])
                nc.vector.tensor_sub(fnb[0:TP, :], h[:], mq[:])
                # fp8 recast of fn' for next matmuls / transposes
                nc.vector.tensor_copy(fr[:, 0, :], fnb[0:TP, :])

                # output: o = Woa@fn' (+ ox) -> f16
                o_ps = psO.tile([TP, N], F32, name="o_ps", tag="o")
                nc.tensor.matmul(o_ps[:], woa[:], fnb[0:TP, :],
                                 start=True, stop=True)
                nc.gpsimd.tensor_add(osb[:, t, :], o_ps[:], ox[:])
                if t == T - 1:
                    nc.sync.dma_start(out_d.ap()[i], osb[:])

                # fn' -> L1 for next aggregation (fp8 PE transposes)
                if t < T - 1:
                    tp_ps = psA.tile([128, MK, TP], FP8, name="tp_ps", tag="tp")
                    fnl1 = spool.tile([128, MK, TP], FP8, name="fnl1_sb")
                    for k in range(MK):
                        nc.tensor.transpose(
                            tp_ps[:, k, :],
                            fr[:, 0, 128*k:128*(k+1)],
                            ident[0:TP, 0:TP])
                    nc.scalar.copy(fnl1[:], tp_ps[:])
                    st[i]["fnl1"] = fnl1

            for w in range(NT + 2 * WSTRIDE + 1):
                for t in range(T):
                    i = w - t * WSTRIDE
                    if 0 <= i < NT:
                        emit_step(i, t)

    nc.compile()
    return nc


_NC_CACHE = None


def _get_nc():
    global _NC_CACHE
    if _NC_CACHE is None:
        _NC_CACHE = build_nc()
    return _NC_CACHE


def _q8(x, scale=1.0):
    return np.clip(np.asarray(x, np.float32) * scale, -240, 240).astype(E4NP)


def _kron6(w):
    return np.kron(np.eye(BPER, dtype=np.float32), np.asarray(w, np.float32).T)


def _host_prep(x, A_in, W3w, W3u, W4w, W4u, W5w, W5u, W_out, b_out):
    f32 = np.float32
    A_in = np.asarray(A_in, f32)

    def achunks(a):  # [N,N] -> [128, MK, N], m = 128*k + p, fp8 scaled
        return np.ascontiguousarray(
            _q8(a, SA).reshape(MK, 128, N).transpose(1, 0, 2))

    def pair8(wa, wb, s):  # [K,2,M] fp8 DoubleRow stationary
        p = np.stack([_kron6(wa), _kron6(wb)], axis=1)   # [120, 2, 120]
        return np.ascontiguousarray(_q8(p, s))

    zeros = np.zeros((H, H), f32)
    shared = {
        "atk": achunks(A_in.T),
        "ak": achunks(A_in),
        "wz_ag": pair8(W3w[:, :H], W3w[:, H:], SW),
        "wr_ag": pair8(-W4w[:, :H], -W4w[:, H:], SW),
        "wh_ag": pair8(W5w[:, :H], W5w[:, H:], SW),
        "wz_fn": pair8(W3u, zeros, SG),
        "wr_fn": pair8(-W4u, zeros, SG),
        "woa": np.ascontiguousarray(_kron6(W_out[:, :H]).astype(BFNP)),
        "wob": np.ascontiguousarray(np.concatenate(
            [_kron6(W_out[:, H:]),
             np.tile(np.asarray(b_out, f32), BPER)[None, :]], axis=0
        ).astype(BFNP)),
        "whu": np.ascontiguousarray((_kron6(W5u) * SG).astype(BFNP)),
        "ident": np.eye(128, dtype=f32).astype(E4NP),
    }

    in_maps = []
    x = np.asarray(x, f32)
    for c in range(NCORES):
        xp = np.zeros((BPAD, N, H), f32)
        xp[:BS] = x[BS * c:BS * (c + 1)]
        # L1: [m, (b,h)] -> [NT, 128(p), MK(k), TP], m = 128k+p
        l1 = xp.transpose(1, 0, 2).reshape(N, NT, TP).transpose(1, 0, 2)
        l1 = l1.reshape(NT, MK, 128, TP).transpose(0, 2, 1, 3)
        # L2: [(b,h), n] -> [NT, TP, N]
        l2 = xp.transpose(0, 2, 1).reshape(NT, TP, N)
        l2e = np.concatenate(
            [l2, np.ones((NT, 1, N), f32)], axis=1)     # ones row for bias
        in_maps.append({
            "xl1": np.ascontiguousarray(_q8(l1)),
            "xbf": np.ascontiguousarray(l2e.astype(BFNP)),
            "xfp": np.ascontiguousarray(_q8(l2)),
            **shared})
    return in_maps


def kernel(x, A_in, W3w, W3u, W4w, W4u, W5w, W5u, W_out, b_out):
    global LAST_RESULTS
    nc = _get_nc()
    in_maps = _host_prep(x, A_in, W3w, W3u, W4w, W4u, W5w, W5u, W_out, b_out)
    res = run_bass_kernel_spmd(nc, in_maps, core_ids=list(range(NCORES)))
    LAST_RESULTS = res
    outs = []
    for c in range(NCORES):
        o = np.asarray(res.results[c]["out"], np.float32)  # [NT, TP, T, N]
        o = o.reshape(NT, BPER, H, T, N).transpose(3, 0, 1, 4, 2)
        outs.append(o.reshape(T, BPAD, N, H)[:, :BS])
    return np.ascontiguousarray(np.concatenate(outs, axis=1))
